# revision 10
# baseline (speedup 1.0000x reference)
"""LoRA layer (x @ W.T + (x@A)@B + bias) on 8 trn2 NeuronCores.

Data-parallel: core b computes batch b's (2048, 4096) output slice.
The low-rank path is folded on the host (W_eff.T = W.T + A@B, a rank-16
update, 0.4% of the FLOPs), so the device kernel is a pure GEMM + bias.
GEMM operands are bf16: matmul streaming is 1 column/cycle for both
bf16 and fp32r, but bf16 gets 2x faster FWL weight loads and halves
DMA traffic + SBUF footprint (x fits resident); rel-err ~4e-3, well
under the 2e-2 gate.

Per-core schedule: x^T (bf16, 128 KB/partition) is fully resident in
SBUF; W_eff^T streams through once (32 MiB). For each 512-wide output
column group and each 128-col stationary W block, the full 2048-token
sequence streams through 4 PSUM banks (4 matmuls of N=512 per weight
load, double-buffered 4+4 across blocks). Tile emits a redundant
LDWEIGHTS per matmul; _dedup_ldweights removes them so one weight load
serves all 4 matmuls (LDWEIGHTS can't overlap in-flight matmuls, so
each redundant load costs ~53ns of PE time — ~18% of the kernel).
Bias is added during the PSUM->SBUF eviction (per-partition
tensor_scalar add) and the output is DMA'd transposed ([DOUT, SEQ]
bf16) and re-transposed/upcast on the host. The first column-group
pass instead does 8 matmuls per x k-tile (halved x consumption rate)
so the PE never outruns the initial 16 MiB x DMA stream. Measured
~891-900 us/core = ~97% of the 874 us PE streaming floor (4096 N=512
matmuls at 1 column/cycle, 2.4 GHz warm).
"""
import contextlib

import ml_dtypes
import numpy as np

import concourse.mybir as mybir
import concourse.tile as tile
from concourse import bacc
from concourse.bass_utils import run_bass_kernel_spmd

BATCH, SEQ, DIN, DOUT, RANK = 8, 2048, 4096, 4096, 16
N_CORES = 8

KT = DIN // 128          # 32 contraction tiles
OG = DOUT // 512         # 8 output column groups
SC = SEQ // 1024         # 2 token super-chunks
BF16 = mybir.dt.bfloat16
F32 = mybir.dt.float32
NP_BF16 = ml_dtypes.bfloat16

_nc_cache = {}


def _dedup_ldweights(nc):
    """Remove InstLdweights whose weights AP equals the previous kept
    InstLdweights with no intervening PE-array-state change, merging any
    semaphore waits/updates into the next kept instruction. Tile emits a
    redundant LDWEIGHTS per matmul; the PE array keeps the stationary
    operand between matmuls, so one load serves the whole run (the
    full-array LDWEIGHTS cannot overlap in-flight matmuls, so each
    redundant load costs ~53ns of PE time)."""
    removed = 0
    for fn in nc.m.functions:
        for blk in fn.blocks:
            out = []
            last_key = None
            pend_w, pend_u = [], []
            for inst in blk.instructions:
                nm = type(inst).__name__
                if nm == "InstLdweights":
                    key = str(inst.ins[0])
                    if key == last_key:
                        si = inst.sync_info
                        if si is not None:
                            pend_w.extend(list(si.on_wait))
                            pend_u.extend(list(si.on_update))
                        removed += 1
                        continue
                    last_key = key
                elif nm == "InstMatmult":
                    if inst.is_transpose or inst.ldweights is not False:
                        last_key = None
                if pend_w or pend_u:
                    si = inst.sync_info
                    w = list(si.on_wait) if si is not None else []
                    u = list(si.on_update) if si is not None else []
                    inst.sync_info = mybir.SyncInfo(
                        on_wait=w + pend_w, on_update=u + pend_u)
                    pend_w, pend_u = [], []
                out.append(inst)
            assert not pend_w and not pend_u, "dangling sync at block end"
            blk.instructions = out
    return removed


def build(reps=1):
    nc = bacc.Bacc("TRN2", target_bir_lowering=False, debug=False)
    xT = nc.dram_tensor("xT", [DIN, SEQ], BF16, kind="ExternalInput")
    wT = nc.dram_tensor("wT", [DIN, DOUT], BF16, kind="ExternalInput")
    biasT = nc.dram_tensor("biasT", [128, DOUT // 128], F32, kind="ExternalInput")
    outT = nc.dram_tensor("outT", [DOUT, SEQ], BF16, kind="ExternalOutput")

    with tile.TileContext(nc) as tc:
        with (
            tc.tile_pool(name="xblk", bufs=KT) as xpool,
            tc.tile_pool(name="wt", bufs=KT + 8) as wpool,
            tc.tile_pool(name="bias", bufs=1) as bpool,
            tc.tile_pool(name="outp", bufs=8) as opool,
            tc.tile_pool(name="psum", bufs=8, space="PSUM") as ppool,
        ):
            bias_sb = bpool.tile([128, DOUT // 128], F32, tag="bias")
            nc.sync.dma_start(bias_sb[:], biasT[:, :])

            rep_ctx = tc.For_i(0, reps, 1) if reps > 1 else contextlib.nullcontext()
            with rep_ctx:
                xtiles = [None] * KT

                def get_x(k):
                    if xtiles[k] is None:
                        xt = xpool.tile([128, SEQ], BF16, name="x", tag="x")
                        nc.sync.dma_start(xt[:], xT[k * 128:(k + 1) * 128, :])
                        xtiles[k] = xt
                    return xtiles[k]

                def evict(ps, c0, t0, bcol):
                    ot = opool.tile([128, 512], BF16, name="o", tag="o")
                    nc.vector.tensor_scalar_add(
                        ot[:], ps[:], bias_sb[:, bcol:bcol + 1])
                    nc.sync.dma_start(outT[c0:c0 + 128, t0:t0 + 512], ot[:])

                for og in range(OG):
                    og0 = og * 512
                    wts = []
                    for k in range(KT):
                        wt_t = wpool.tile([128, 512], BF16, name="w", tag="w")
                        nc.sync.dma_start(
                            wt_t[:], wT[k * 128:(k + 1) * 128, og0:og0 + 512])
                        wts.append(wt_t)
                    if og == 0:
                        # First pass streams x in from HBM: 8 matmuls per
                        # x k-tile (halved consumption rate) so the PE never
                        # outruns the 16 MiB x DMA stream.
                        for sc in range(SC):
                            s0 = sc * 1024
                            psums = [ppool.tile([128, 512], F32, name="ps",
                                                tag="ps") for _ in range(8)]
                            for k in range(KT):
                                xt = get_x(k)
                                for oi in range(4):
                                    for mc in range(2):
                                        nc.tensor.matmul(
                                            psums[oi * 2 + mc][:],
                                            wts[k][:, oi * 128:(oi + 1) * 128],
                                            xt[:, s0 + mc * 512:s0 + (mc + 1) * 512],
                                            start=(k == 0), stop=(k == KT - 1))
                            for oi in range(4):
                                for mc in range(2):
                                    evict(psums[oi * 2 + mc], og0 + oi * 128,
                                          s0 + mc * 512, og * 4 + oi)
                    else:
                        # Steady state: one stationary 128x128 W block
                        # streams all 2048 tokens (4 matmuls per weight
                        # load after _dedup_ldweights), 4+4 PSUM banks
                        # double-buffered across blocks.
                        for oi in range(4):
                            psums = [ppool.tile([128, 512], F32, name="ps",
                                                tag="ps") for _ in range(4)]
                            for k in range(KT):
                                xt = get_x(k)
                                for t in range(4):
                                    nc.tensor.matmul(
                                        psums[t][:],
                                        wts[k][:, oi * 128:(oi + 1) * 128],
                                        xt[:, t * 512:(t + 1) * 512],
                                        start=(k == 0), stop=(k == KT - 1))
                            for t in range(4):
                                evict(psums[t], og0 + oi * 128, t * 512,
                                      og * 4 + oi)
    _dedup_ldweights(nc)
    nc.compile()
    return nc


def prepare_inputs(x, A, B, weight, bias):
    x = np.asarray(x, dtype=np.float32)
    A = np.asarray(A, dtype=np.float32)
    B = np.asarray(B, dtype=np.float32)
    weight = np.asarray(weight, dtype=np.float32)
    bias = np.asarray(bias, dtype=np.float32)

    wT_eff = (weight.T + A @ B).astype(NP_BF16)              # [DIN, DOUT]
    biasT = np.ascontiguousarray(
        bias.reshape(DOUT // 128, 128).T)                    # [128, 32]

    in_maps = []
    for b in range(N_CORES):
        xTb = np.ascontiguousarray(x[b].astype(NP_BF16).T)   # [DIN, SEQ]
        in_maps.append({"xT": xTb, "wT": wT_eff, "biasT": biasT})
    return in_maps


def assemble(results):
    return np.stack(
        [np.ascontiguousarray(r["outT"].astype(np.float32).T)
         for r in results], axis=0)


def kernel(x, A, B, weight, bias):
    if 1 not in _nc_cache:
        _nc_cache[1] = build(reps=1)
    nc = _nc_cache[1]
    in_maps = prepare_inputs(x, A, B, weight, bias)
    res = run_bass_kernel_spmd(nc, in_maps, core_ids=list(range(N_CORES)))
    last_result.clear()
    last_result.append(res)
    return assemble(res.results)


last_result = []


# revision 12
# speedup vs baseline: 1.0502x; 1.0502x over previous
"""LoRA layer (x @ W.T + (x@A)@B + bias) on 8 trn2 NeuronCores.

Data-parallel: core b computes batch b's (2048, 4096) output slice.
The low-rank path is folded on the host (W_eff.T = W.T + A@B, a rank-16
update, 0.4% of the FLOPs), so the device kernel is a pure GEMM + bias.
GEMM operands are bf16: matmul streaming is 1 column/cycle for both
bf16 and fp32r, but bf16 gets 2x faster FWL weight loads and halves
DMA traffic + SBUF footprint (x fits resident); rel-err ~4e-3, well
under the 2e-2 gate.

Per-core schedule: x^T (bf16, 128 KB/partition) is fully resident in
SBUF; W_eff^T streams through once (32 MiB). For each 512-wide output
column group and each 128-col stationary W block, the full 2048-token
sequence streams through 4 PSUM banks (4 matmuls of N=512 per weight
load, double-buffered 4+4 across blocks). Tile emits a redundant
LDWEIGHTS per matmul; _dedup_ldweights removes them so one weight load
serves all 4 matmuls (LDWEIGHTS can't overlap in-flight matmuls, so
each redundant load costs ~53ns of PE time — ~18% of the kernel).
Bias is added during the PSUM->SBUF eviction (per-partition
tensor_scalar add) and the output is DMA'd transposed ([DOUT, SEQ]
bf16) and re-transposed/upcast on the host. The first column-group
pass instead runs 2 col-groups of 256 over the full token sweep —
8 matmuls per x k-tile (halved x consumption rate) so the PE never
outruns the initial 16 MiB x DMA stream, while keeping weight-load
runs of 4. Measured ~885 us/core interleaved-A/B (~8% faster than the
plain full-resident schedule) = ~97-99% of the 874 us PE streaming
floor (4096 N=512 matmuls at 1 column/cycle, 2.4 GHz warm).
"""
import contextlib

import ml_dtypes
import numpy as np

import concourse.mybir as mybir
import concourse.tile as tile
from concourse import bacc
from concourse.bass_utils import run_bass_kernel_spmd

BATCH, SEQ, DIN, DOUT, RANK = 8, 2048, 4096, 4096, 16
N_CORES = 8

KT = DIN // 128          # 32 contraction tiles
OG = DOUT // 512         # 8 output column groups
SC = SEQ // 1024         # 2 token super-chunks
BF16 = mybir.dt.bfloat16
F32 = mybir.dt.float32
NP_BF16 = ml_dtypes.bfloat16

_nc_cache = {}


def _dedup_ldweights(nc):
    """Remove InstLdweights whose weights AP equals the previous kept
    InstLdweights with no intervening PE-array-state change, merging any
    semaphore waits/updates into the next kept instruction. Tile emits a
    redundant LDWEIGHTS per matmul; the PE array keeps the stationary
    operand between matmuls, so one load serves the whole run (the
    full-array LDWEIGHTS cannot overlap in-flight matmuls, so each
    redundant load costs ~53ns of PE time)."""
    removed = 0
    for fn in nc.m.functions:
        for blk in fn.blocks:
            out = []
            last_key = None
            pend_w, pend_u = [], []
            for inst in blk.instructions:
                nm = type(inst).__name__
                if nm == "InstLdweights":
                    key = str(inst.ins[0])
                    if key == last_key:
                        si = inst.sync_info
                        if si is not None:
                            pend_w.extend(list(si.on_wait))
                            pend_u.extend(list(si.on_update))
                        removed += 1
                        continue
                    last_key = key
                elif nm == "InstMatmult":
                    if inst.is_transpose or inst.ldweights is not False:
                        last_key = None
                if pend_w or pend_u:
                    si = inst.sync_info
                    w = list(si.on_wait) if si is not None else []
                    u = list(si.on_update) if si is not None else []
                    inst.sync_info = mybir.SyncInfo(
                        on_wait=w + pend_w, on_update=u + pend_u)
                    pend_w, pend_u = [], []
                out.append(inst)
            assert not pend_w and not pend_u, "dangling sync at block end"
            blk.instructions = out
    return removed


def build(reps=1):
    nc = bacc.Bacc("TRN2", target_bir_lowering=False, debug=False)
    xT = nc.dram_tensor("xT", [DIN, SEQ], BF16, kind="ExternalInput")
    wT = nc.dram_tensor("wT", [DIN, DOUT], BF16, kind="ExternalInput")
    biasT = nc.dram_tensor("biasT", [128, DOUT // 128], F32, kind="ExternalInput")
    outT = nc.dram_tensor("outT", [DOUT, SEQ], BF16, kind="ExternalOutput")

    with tile.TileContext(nc) as tc:
        with (
            tc.tile_pool(name="xblk", bufs=KT) as xpool,
            tc.tile_pool(name="wt", bufs=KT + 8) as wpool,
            tc.tile_pool(name="bias", bufs=1) as bpool,
            tc.tile_pool(name="outp", bufs=8) as opool,
            tc.tile_pool(name="psum", bufs=8, space="PSUM") as ppool,
        ):
            bias_sb = bpool.tile([128, DOUT // 128], F32, tag="bias")
            nc.sync.dma_start(bias_sb[:], biasT[:, :])

            rep_ctx = tc.For_i(0, reps, 1) if reps > 1 else contextlib.nullcontext()
            with rep_ctx:
                xtiles = [None] * KT

                def get_x(k):
                    if xtiles[k] is None:
                        xt = xpool.tile([128, SEQ], BF16, name="x", tag="x")
                        nc.sync.dma_start(xt[:], xT[k * 128:(k + 1) * 128, :])
                        xtiles[k] = xt
                    return xtiles[k]

                def evict(ps, c0, t0, bcol):
                    ot = opool.tile([128, 512], BF16, name="o", tag="o")
                    nc.vector.tensor_scalar_add(
                        ot[:], ps[:], bias_sb[:, bcol:bcol + 1])
                    nc.sync.dma_start(outT[c0:c0 + 128, t0:t0 + 512], ot[:])

                for og in range(OG):
                    og0 = og * 512
                    wts = []
                    for k in range(KT):
                        wt_t = wpool.tile([128, 512], BF16, name="w", tag="w")
                        nc.sync.dma_start(
                            wt_t[:], wT[k * 128:(k + 1) * 128, og0:og0 + 512])
                        wts.append(wt_t)
                    if og == 0:
                        # First pass streams x in from HBM: 2 col-groups of
                        # 256 over the full token sweep — 8 matmuls per
                        # x k-tile (halved consumption rate) so the PE never
                        # outruns the 16 MiB x DMA stream, while keeping
                        # LDWEIGHTS runs of 4.
                        for go2 in range(2):
                            psums = [ppool.tile([128, 512], F32, name="ps",
                                                tag="ps") for _ in range(8)]
                            for k in range(KT):
                                xt = get_x(k)
                                for oi in range(2):
                                    w_off = go2 * 256 + oi * 128
                                    for t in range(4):
                                        nc.tensor.matmul(
                                            psums[oi * 4 + t][:],
                                            wts[k][:, w_off:w_off + 128],
                                            xt[:, t * 512:(t + 1) * 512],
                                            start=(k == 0), stop=(k == KT - 1))
                            for oi in range(2):
                                for t in range(4):
                                    evict(psums[oi * 4 + t],
                                          go2 * 256 + oi * 128, t * 512,
                                          go2 * 2 + oi)
                    else:
                        # Steady state: one stationary 128x128 W block
                        # streams all 2048 tokens (4 matmuls per weight
                        # load after _dedup_ldweights), 4+4 PSUM banks
                        # double-buffered across blocks.
                        for oi in range(4):
                            psums = [ppool.tile([128, 512], F32, name="ps",
                                                tag="ps") for _ in range(4)]
                            for k in range(KT):
                                xt = get_x(k)
                                for t in range(4):
                                    nc.tensor.matmul(
                                        psums[t][:],
                                        wts[k][:, oi * 128:(oi + 1) * 128],
                                        xt[:, t * 512:(t + 1) * 512],
                                        start=(k == 0), stop=(k == KT - 1))
                            for t in range(4):
                                evict(psums[t], og0 + oi * 128, t * 512,
                                      og * 4 + oi)
    _dedup_ldweights(nc)
    nc.compile()
    return nc


def prepare_inputs(x, A, B, weight, bias):
    x = np.asarray(x, dtype=np.float32)
    A = np.asarray(A, dtype=np.float32)
    B = np.asarray(B, dtype=np.float32)
    weight = np.asarray(weight, dtype=np.float32)
    bias = np.asarray(bias, dtype=np.float32)

    wT_eff = (weight.T + A @ B).astype(NP_BF16)              # [DIN, DOUT]
    biasT = np.ascontiguousarray(
        bias.reshape(DOUT // 128, 128).T)                    # [128, 32]

    in_maps = []
    for b in range(N_CORES):
        xTb = np.ascontiguousarray(x[b].astype(NP_BF16).T)   # [DIN, SEQ]
        in_maps.append({"xT": xTb, "wT": wT_eff, "biasT": biasT})
    return in_maps


def assemble(results):
    return np.stack(
        [np.ascontiguousarray(r["outT"].astype(np.float32).T)
         for r in results], axis=0)


def kernel(x, A, B, weight, bias):
    if 1 not in _nc_cache:
        _nc_cache[1] = build(reps=1)
    nc = _nc_cache[1]
    in_maps = prepare_inputs(x, A, B, weight, bias)
    res = run_bass_kernel_spmd(nc, in_maps, core_ids=list(range(N_CORES)))
    last_result.clear()
    last_result.append(res)
    return assemble(res.results)


last_result = []
